# revision 29
# baseline (speedup 1.0000x reference)
"""Quantized (4-bit) LoRA linear for Trainium2, SPMD over 8 NeuronCores.

Math:  y[t,o] = sum_i x[t,i]*W[o,i] + bias[o] + 2.0 * sum_r (x@A^T)[t,r]*B[o,r]
where  W[o,i] = (nib[o,i] - zero[i]) * scale[i],  nib = unpacked 4-bit ints.

Strategy (fp8 DoubleRow): with xsF[t,i] = x[t,i]*scale[i]*F (F=128 lifts
values out of the fp8 subnormal range), split xsF = hi + lo into two
fp8e4m3 operands (error-feedback split).  Then

  F*y[t,o] = sum_i (hi+lo)[t,i]*nib[o,i]   (fp8 DoubleRow matmuls; nib in
                                            {0..15} is exact in fp8)
           + G[t,:] @ H[:,o]               (folded into the LAST DoubleRow
                                            inst of each strip -- see below)
           - c_h[t]                        (fused into the DVE eviction as a
                                            per-partition scalar)
  then *(1/F), fp16 store (cast to fp32 on host).

  G rows: 0 = c_l/64 = sum_i xsF*z_lo/64, 1-8 = F*(x@A^T)/64, 9 = 1 (bias)
  H rows: 0 = -64,    1-8 = 128*B^T,          9 = F*bias
  zero = z_hi + z_lo is an fp8 error-feedback split; c_h = sum_i xsF*z_hi is
  extracted to token-partitions via a one-hot matmul into spare u-PSUM cols.

Correction folding: instruction cost scales with OUTPUT columns only, so the
G@H term rides for free inside the last (j=15, lo) DoubleRow inst of every
strip: rows k=4064..4073 (partitions 96-105, pair1 of the last k-tile)
sacrifice their lo-residuals; at runtime a selector matmul + partition-
aligned DVE copy plants the G rows there (const row baked statically), and
a host-built hybrid nib tile carries the matching H columns.  The matching
(j15, lo) u-instruction uses a zeroed aext variant so the planted rows
contribute nothing to u/c.  Cost of the dropped lo rows: ~ +4e-4 rel err.

DoubleRow (both operands fp8e4m3, max 240!) contracts 2 k-tiles (256 rows)
per instruction at 0.5 cycles/row = 4x the fp16 matmul rate; hi+lo -> 2x.

Schedule: 8-way token split (1024 tokens/core).  Work unit = one PSUM strip
[128 tokens x 512 outs] accumulating all 4096 contraction rows in 64
DoubleRow insts.  Phase A streams xs (8 MB) + the first o-strip column of
nib j-major with 6 strips + 2 u-chain banks in flight, pacing the PE right
behind the DMA.  Steady state runs strips chain-major (nib o-columns of 512
stream 16 tiles each, prefetched one column ahead), each strip's eviction
trailing one strip behind so PSUM bank reuse never stalls the PE.
Eviction is one DVE op: (psum - c_h[t]) * (1/F) -> fp16.
"""

import numpy as np

B, S, I, O = 4, 2048, 4096, 4096
T = B * S            # 8192 tokens
NCORES = 8
TC = T // NCORES     # 1024 tokens per core
KP = I // 256        # 16 k-pairs (DoubleRow contracts 256 rows each)
OH = O // 2          # o-half width
NTT = TC // 128      # 8 token tiles per core
NS = OH // 512       # 4 psum strips per o-half
F = 128.0            # fp8 pre-scale (power of two, exact)

_CACHE = {}


def _build_program():
    import concourse.bacc as bacc
    import concourse.mybir as mybir
    import concourse.tile as tile

    fp16 = mybir.dt.float16
    fp32 = mybir.dt.float32
    fp8 = mybir.dt.float8e4
    DR = mybir.MatmulPerfMode.DoubleRow

    nc = bacc.Bacc("TRN2", target_bir_lowering=False, debug=False)
    xs8 = nc.dram_tensor("xs8", [KP, 128, 2, 2, TC], fp8, kind="ExternalInput")
    nib8 = nc.dram_tensor("nib8", [KP, 128, 2, O], fp8, kind="ExternalInput")
    aext8 = nc.dram_tensor("aext8", [128, KP + 1, 2, 16], fp8, kind="ExternalInput")
    e9 = nc.dram_tensor("e9", [16, 1], fp16, kind="ExternalInput")
    sel16 = nc.dram_tensor("sel16", [16, 128], fp16, kind="ExternalInput")
    nibh8 = nc.dram_tensor("nibh8", [128, 2, O], fp8, kind="ExternalInput")
    y = nc.dram_tensor("y", [TC, O], fp16, kind="ExternalOutput")

    with tile.TileContext(nc) as tc:
        with (
            tc.tile_pool(name="xs", bufs=1) as xs_pool,
            tc.tile_pool(name="nib", bufs=22) as nib_pool,
            tc.tile_pool(name="consts", bufs=1) as const_pool,
            tc.tile_pool(name="g", bufs=1) as g_pool,
            tc.tile_pool(name="out", bufs=3) as out_pool,
            tc.tile_pool(name="psum", bufs=8, space="PSUM") as psum_pool,
        ):
            uf16 = g_pool.tile([16, TC], fp16, tag="uf", name="uf16")
            e9_tile = const_pool.tile([16, 1], fp16, tag="e9")
            sel_tile = const_pool.tile([16, 128], fp16, tag="sel")
            ct32all = g_pool.tile([128, NTT], fp32, tag="ct", name="ct32all")
            xs_tiles = [None] * KP
            aext_all = const_pool.tile([128, KP + 1, 2, 16], fp8, tag="aext")
            nib_tiles = {}  # (j, half) -> tile
            nc.vector.memset(gt8[:, :, :], 0.0)
            nc.vector.memset(gt8[:, 0, :], 1.0)  # const row (bias)

            def load_nib(j, h):
                nt = nib_pool.tile([128, 2, OH], fp8, tag="nib",
                                   name=f"nib{h}_{j}")
                nc.sync.dma_start(nt[:], nib8[j][:, :, h * OH:(h + 1) * OH])
                nib_tiles[(j, h)] = nt

            def main_insts(j, h, tt, ps, first):
                """The 4 DoubleRow insts of k-pair j for chain (h,tt,s)."""
                for s in range(NS):
                    for hl in range(2):
                        for c in range(2):
                            nc.tensor.matmul(
                                ps[s][:, c * 256:(c + 1) * 256],
                                xs_tiles[j][:, :, hl,
                                            tt * 128:(tt + 1) * 128],
                                nib_tiles[(j, h)][:, :,
                                                  s * 512 + c * 256:
                                                  s * 512 + (c + 1) * 256],
                                start=(first and hl == 0 and c == 0),
                                stop=False, perf_mode=DR,
                            )

            def finish_tt(h, tt, ps):
                """LoRA/zero/bias matmul + eviction + store for chain group."""
                ot = out_pool.tile([128, OH], fp16, tag="out",
                                   name=f"ot{h}_{tt}")
                for s in range(NS):
                    for c in range(2):
                        off = h * OH + s * 512 + c * 256
                        nc.tensor.matmul(
                            ps[s][:, c * 256:(c + 1) * 256],
                            gt8[:, :, tt * 128:(tt + 1) * 128],
                            h_tile[:, :, off:off + 256],
                            start=False, stop=(c == 1), perf_mode=DR,
                        )
                for s in range(NS):
                    nc.vector.tensor_scalar(
                        ot[:, s * 512:(s + 1) * 512], ps[s][:, :],
                        ct32_tiles[tt][:, :], 1.0 / F,
                        op0=mybir.AluOpType.subtract,
                        op1=mybir.AluOpType.mult)
                nc.scalar.dma_start(
                    y[tt * 128:(tt + 1) * 128, h * OH:(h + 1) * OH], ot[:])

            def new_ps(h, tt):
                return [psum_pool.tile([128, 512], fp32, tag="mm",
                                       name=f"ps{h}_{tt}_{s}")
                        for s in range(NS)]

            # ---- group 0 (half 0, tt 0 + first half of tt 1) + u chains ----
            # j-major emission right behind the streaming DMAs; 8 PSUM banks
            # all open (2 u + 4 tt0 + 2 tt1) to maximize PE fill while the
            # 16 MB of phase-A DMA (xs + nib half 0) streams in.
            up = [psum_pool.tile([16, 512], fp32, tag="mm", name=f"up{uc}")
                  for uc in range(2)]
            ps_prev = new_ps(0, 0)
            ps_tt1 = new_ps(0, 1)  # s0/s1 filled in group 0, s2/s3 in group 1
            for j in range(KP):
                xt = xs_pool.tile([128, 2, 2, TC], fp8, tag=f"xs{j}",
                                  name=f"xs{j}")
                nc.sync.dma_start(xt[:], xs8[j])
                xs_tiles[j] = xt
                if j == 0:
                    nc.sync.dma_start(aext_all[:], aext8[:, :, :, :])
                    nc.sync.dma_start(e9_tile[:], e9[:, :])
                    nc.sync.dma_start(sel_tile[:], sel16[:, :])
                if j == 1:
                    load_nibh(0)
                load_nib(j, 0)
                for uc in range(2):
                    for hl in range(2):
                        for c in range(2):
                            nc.tensor.matmul(
                                up[uc][:, c * 256:(c + 1) * 256],
                                aext_all[:, j, :, :],
                                xs_tiles[j][:, :, hl,
                                            uc * 512 + c * 256:
                                            uc * 512 + (c + 1) * 256],
                                start=(j == 0 and hl == 0 and c == 0),
                                stop=(j == KP - 1 and hl == 1 and c == 1),
                                perf_mode=DR,
                            )
                main_insts(j, 0, 0, ps_prev, first=(j == 0))
                for s in range(2):
                    for hl in range(2):
                        for c in range(2):
                            nc.tensor.matmul(
                                ps_tt1[s][:, c * 256:(c + 1) * 256],
                                xs_tiles[j][:, :, hl, 128:256],
                                nib_tiles[(j, 0)][:, :,
                                                  s * 512 + c * 256:
                                                  s * 512 + (c + 1) * 256],
                                start=(j == 0 and hl == 0 and c == 0),
                                stop=False, perf_mode=DR,
                            )
            for uc in range(2):
                nc.vector.tensor_copy(uf16[:, uc * 512:(uc + 1) * 512],
                                      up[uc][:, :])
            for tt in range(NTT):
                nc.scalar.dma_start_transpose(
                    ct_tiles[tt][:, :], uf16[:, tt * 128:(tt + 1) * 128])
            for tt in range(NTT):
                nc.gpsimd.tensor_copy(ct32_tiles[tt][:, :],
                                      ct_tiles[tt][:, 9:10])
            # early prefetch of half-1 nib into the 6 spare pool slots
            for j in range(6):
                load_nib(j, 1)
            # group 1: finish tt1 (strips 2,3 only)
            for j in range(KP):
                for s in (2, 3):
                    for hl in range(2):
                        for c in range(2):
                            nc.tensor.matmul(
                                ps_tt1[s][:, c * 256:(c + 1) * 256],
                                xs_tiles[j][:, :, hl, 128:256],
                                nib_tiles[(j, 0)][:, :,
                                                  s * 512 + c * 256:
                                                  s * 512 + (c + 1) * 256],
                                start=(j == 0 and hl == 0 and c == 0),
                                stop=False, perf_mode=DR,
                            )
            finish_tt(0, 0, ps_prev)
            ps_prev, tt_prev = ps_tt1, 1

            # ---- remaining groups, h/evict work staggered one group back ---
            for h in range(2):
                for tt in range(NTT):
                    if h == 0 and tt <= 1:
                        continue
                    if h == 1 and tt == NTT - 1:
                        break  # last group handled strip-major below
                    ps = new_ps(h, tt)
                    for j in range(KP):
                        if h == 1 and tt == 0 and j >= 6:
                            load_nib(j, 1)
                        main_insts(j, h, tt, ps, first=(j == 0))
                    finish_tt(h if tt > 0 else 0, tt_prev, ps_prev)
                    ps_prev, tt_prev = ps, tt

            # ---- last group (h=1, tt=7): strip-major so the tail drains
            # strip-by-strip instead of all-at-once after the final matmul.
            h, tt = 1, NTT - 1
            ps = new_ps(h, tt)
            ot_last = out_pool.tile([128, OH], fp16, tag="out", name="ot_last")
            for s in range(NS):
                for j in range(KP):
                    for hl in range(2):
                        for c in range(2):
                            nc.tensor.matmul(
                                ps[s][:, c * 256:(c + 1) * 256],
                                xs_tiles[j][:, :, hl,
                                            tt * 128:(tt + 1) * 128],
                                nib_tiles[(j, h)][:, :,
                                                  s * 512 + c * 256:
                                                  s * 512 + (c + 1) * 256],
                                start=(j == 0 and hl == 0 and c == 0),
                                stop=False, perf_mode=DR,
                            )
                if s == 0:
                    finish_tt(1, tt_prev, ps_prev)
                for c in range(2):
                    off = h * OH + s * 512 + c * 256
                    nc.tensor.matmul(
                        ps[s][:, c * 256:(c + 1) * 256],
                        gt8[:, :, tt * 128:(tt + 1) * 128],
                        h_tile[:, :, off:off + 256],
                        start=False, stop=(c == 1), perf_mode=DR,
                    )
                nc.vector.tensor_scalar(
                    ot_last[:, s * 512:(s + 1) * 512], ps[s][:, :],
                    ct32_tiles[tt][:, :], 1.0 / F,
                    op0=mybir.AluOpType.subtract, op1=mybir.AluOpType.mult)
                nc.scalar.dma_start(
                    y[tt * 128:(tt + 1) * 128,
                      h * OH + s * 512:h * OH + (s + 1) * 512],
                    ot_last[:, s * 512:(s + 1) * 512])
    nc.compile()
    return nc


def _prep_inputs(x, weight_quant, scale, zero, lora_A, lora_B, bias):
    """Host-side layout prep + sharding. Returns in_maps for 8 cores."""
    import ml_dtypes

    e4 = ml_dtypes.float8_e4m3   # device fp8e4: e4m3 WITH inf, max 240

    # xsF = x*scale*F, split hi/lo fp8, laid out [KP, 128, pair, hi/lo, T]
    xsF = (x.reshape(T, I).astype(np.float32) * (scale[None, :] * F))
    hi = xsF.astype(e4)
    lo = (xsF - hi.astype(np.float32)).astype(e4)

    def kshuf(arr):  # [I, ...] -> [KP, 128, 2, ...]
        return np.ascontiguousarray(
            arr.reshape(KP, 2, 128, *arr.shape[1:]).swapaxes(1, 2))

    xs8 = np.empty((KP, 128, 2, 2, T), e4)
    xs8[:, :, :, 0, :] = kshuf(np.ascontiguousarray(hi.T))
    xs8[:, :, :, 1, :] = kshuf(np.ascontiguousarray(lo.T))
    # rows k=4064..4073 (partitions 96-105, pair1 of the last k-tile) host
    # the planted gt rows in their lo slot: drop their lo-residuals, bake
    # the constant bias row (gt row 9 = 1.0) statically
    xs8[KP - 1, 96:105, 1, 1, :] = e4(0.0)
    xs8[KP - 1, 105, 1, 1, :] = e4(1.0)

    wq = weight_quant.astype(np.uint8)           # low byte only is populated
    nib = np.empty((O, I), np.uint8)
    nib[:, 0::2] = wq & 15
    nib[:, 1::2] = wq >> 4
    nib8 = kshuf(np.ascontiguousarray(nib.T).astype(e4))   # [KP,128,2,O]

    # u-matmul columns: 0 = z_lo residual, 1-8 = A/scale, 9 = z_hi
    aext = np.zeros((I, 16), np.float32)
    z_h = zero.astype(e4)
    z_l = (zero - z_h.astype(np.float32)).astype(e4)
    aext[:, 0] = z_l.astype(np.float32)
    aext[:, 1:9] = (lora_A.astype(np.float32) / scale[None, :]).T
    aext[:, 9] = z_h.astype(np.float32)
    aext8 = kshuf(aext.astype(e4)).transpose(1, 0, 2, 3)   # [128,KP,2,16]
    # slot KP: copy of slot KP-1 with the planted-row partitions zeroed in
    # pair 1 -- used by the (j15, lo) u-instruction so the planted gt rows
    # and the static const row contribute nothing to u/c
    aextlo = aext8[:, KP - 1:KP].copy()
    aextlo[96:106, 0, 1, :] = e4(0.0)
    aext8 = np.ascontiguousarray(
        np.concatenate([aext8, aextlo], axis=1))           # [128,KP+1,2,16]

    # H rows matching the planted gt rows [c_l/64, u0..7/64, 1.0]; they
    # ride in the hybrid last nib tile at (partitions 96-105, pair 1)
    hrows = np.zeros((10, O), np.float32)
    hrows[0] = -64.0
    hrows[1:9] = 128.0 * lora_B.T          # 64 * 2 * B^T
    hrows[9] = F * bias                    # const row carries the bias
    assert np.abs(hrows).max() < 240.0
    nibh8 = nib8[KP - 1].copy()            # [128, 2, O]
    nibh8[96:106, 1, :] = hrows.astype(e4)
    nibh8 = np.ascontiguousarray(nibh8)

    e9v = np.zeros((16, 1), np.float16)
    e9v[9, 0] = 1.0
    sel = np.zeros((16, 128), np.float16)
    for r in range(9):
        sel[r, 96 + r] = 1.0 / 64

    in_maps = []
    for c in range(NCORES):
        in_maps.append({
            "xs8": np.ascontiguousarray(xs8[..., c * TC:(c + 1) * TC]),
            "nib8": nib8,
            "aext8": aext8,
            "nibh8": nibh8,
            "e9": e9v,
            "sel16": sel,
        })
    return in_maps


def run_on_cores(in_maps, trace=False):
    from concourse.bass_utils import run_bass_kernel_spmd

    if "nc" not in _CACHE:
        _CACHE["nc"] = _build_program()
    return run_bass_kernel_spmd(
        _CACHE["nc"], in_maps, list(range(NCORES)), trace=trace
    )


def kernel(x, weight_quant, scale, zero, lora_A, lora_B, bias):
    x = np.asarray(x)
    weight_quant = np.asarray(weight_quant)
    scale = np.asarray(scale, np.float32)
    zero = np.asarray(zero, np.float32)
    lora_A = np.asarray(lora_A, np.float32)
    lora_B = np.asarray(lora_B, np.float32)
    bias = np.asarray(bias, np.float32)

    in_maps = _prep_inputs(x, weight_quant, scale, zero, lora_A, lora_B, bias)
    res = run_on_cores(in_maps).results

    out = np.concatenate(
        [res[c]["y"].astype(np.float32) for c in range(NCORES)], axis=0)
    return np.ascontiguousarray(out).reshape(B, S, O)
